# revision 21
# baseline (speedup 1.0000x reference)
"""Trainium2 Bass kernel for nn_ContinousNormalizingFlowRHS.

Computes, for z in R^{B x Z} and scalar time t:
  h0 = tanh(W1*t + B1); h1 = tanh(einsum('knm,km->kn', W2, h0) + B2)
  w_in  = (W3_win  @ h1[0] + b3_win ).reshape(F, Z)
  w_out = (W3_wout @ h1[1] + b3_wout).reshape(F, Z)
  b     =  W3_b    @ h1[2] + b3_b
  gate  = sigmoid(W3_gate @ h1[3] + b3_gate)
  h = tanh(z @ w_in.T + b); dz = (h*gate) @ w_out / F
  trace = ((1-h^2)*gate) @ (sum(w_in*w_out,1)) / F
  out = concat([dz, -trace[:,None]], -1)

Strategy (8 NeuronCores, single SPMD launch):
  Phase 1 (f-sharded): stream W3_win/W3_wout (33.6 MB/core bf16) on the
  sync HWDGE ring only; consume with the PE (stationary-side matvec,
  FWL) for pe_cols f-columns per matrix and the DVE (mul + fold + reduce)
  for the rest.  Engine queues are kept path-pure: Vector only runs the
  DVE path (PE-path drains go scalar-copy -> gpsimd bias-add), so no
  cross-path head-of-line blocking.
  Each core packs its local (w_inT, gate*w_out, sg, b) into a ~129 KB
  blob; one AllGather replicates all slices.
  Stage B (batch-sharded): each core runs its OWN B/8 batch shard
  against the FULL F, accumulating dz / trace in fp32 PSUM across all
  16 f-blocks, and writes its final [Z+1, B/8] output directly.
"""

import sys
import types
import numpy as np
import ml_dtypes

BF = ml_dtypes.bfloat16

# problem sizes (hardcoded per contract)
Z = 128
N = 256
F = 2048
B = 8192
N_CORES = 8

PE_COLS = 160       # per matrix: local f-columns computed on the PE (rest DVE)
CHUNK_R = 5120      # W3 rows per streamed PE chunk ([128, 5120] bf16 tiles)
DVE_CC = 16         # f-columns per DVE chunk (2048 rows)
BC = 512            # batch columns per stage-B chunk (one PSUM bank)


def _ensure_ntff_hook():
    """run_bass_kernel_spmd(trace=True) under axon needs antenv.axon_hooks."""
    if 'antenv.axon_hooks' in sys.modules:
        return
    try:
        from trn_agent_boot.trn_boot import _ntff_profile_via_ctypes
        hook = _ntff_profile_via_ctypes('/opt/axon/libaxon_pjrt.so')
    except Exception:
        hook = None
    try:
        import antenv
    except Exception:
        return
    mod = types.ModuleType('antenv.axon_hooks')
    mod.get_axon_ntff_profile_hook = lambda: hook
    mod.set_axon_ntff_profile_hook = lambda h: None
    sys.modules['antenv.axon_hooks'] = mod
    antenv.axon_hooks = mod


def build_module(n_cores=N_CORES, b=B, f=F, pe_cols=PE_COLS, chunk_r=CHUNK_R,
                 bc=BC, debug=False):
    """Build the Bass module (SPMD program, one per core)."""
    import concourse.tile as tile
    from concourse import bacc, mybir

    F32 = mybir.dt.float32
    BF16 = mybir.dt.bfloat16
    ADD = mybir.AluOpType.add

    fl = f // n_cores            # local f count
    nfb = fl // 128              # local f blocks of 128
    nfb_g = f // 128             # global f blocks of 128
    rows_pe = pe_cols * 128      # rows of W3 handled by the PE
    dve_cols = fl - pe_cols
    rows_dve = dve_cols * 128
    n_pe_chunks = rows_pe // chunk_r
    rpc = chunk_r // 128         # w columns produced per PE chunk
    dcc = DVE_CC                 # f-columns per DVE chunk
    n_dve_chunks = dve_cols // dcc
    bl = b // n_cores            # per-core batch shard
    nbc = bl // bc               # stage-B batch chunks
    assert rows_pe % chunk_r == 0 and dve_cols % dcc == 0

    # blob layout (bf16 elements): w_inT (z,f) | w_outg (fb,f,z) | sg | b
    SZ_A = 128 * fl
    SZ_B = 128 * fl
    SZ_C = fl
    SZ_D = fl
    BLOB = SZ_A + SZ_B + SZ_C + SZ_D
    OF_B, OF_C, OF_D = SZ_A, SZ_A + SZ_B, SZ_A + SZ_B + SZ_C

    nc = bacc.Bacc("TRN2", target_bir_lowering=False, debug=debug,
                   num_devices=n_cores)

    def inp(name, shape, dt):
        return nc.dram_tensor(name, shape, dt, kind="ExternalInput").ap()

    t_ap = inp("t", [128, 1], F32)                  # t replicated
    par_ap = inp("parc", [128, 24], F32)            # w1c | b1c | b2c
    w2t_ap = inp("w2tc", [128, 2048], BF16)
    # PE slabs: one [128, 2*chunk_r] DMA per chunk (both n-halves)
    w3winT_ap = inp("w3winT_sl", [128, 2 * rows_pe], BF16)
    w3woutT_ap = inp("w3woutT_sl", [128, 2 * rows_pe], BF16)
    # DVE slabs: one [128, 2*dcc*N] DMA per pair of compute chunks
    w3winN_ap = inp("w3winN_sl", [rows_dve // (2 * dcc * 128) * 128,
                                  2 * dcc * N], BF16)
    w3woutN_ap = inp("w3woutN_sl", [rows_dve // (2 * dcc * 128) * 128,
                                    2 * dcc * N], BF16)
    b3win_ap = inp("b3win_c", [128, fl], F32)
    b3wout_ap = inp("b3wout_c", [128, fl], F32)
    w3bT_ap = inp("w3bT_sl", [N, fl], BF16)
    w3gateT_ap = inp("w3gateT_sl", [N, fl], BF16)
    b3b_ap = inp("b3b_c", [128, nfb], F32)
    b3gate_ap = inp("b3gate_c", [128, nfb], F32)
    zt_ap = inp("ztb_sl", [128, bl], BF16)          # own batch shard only
    eye_ap = inp("eyeb", [128, 128], BF16)
    out_ap = nc.dram_tensor("out", [Z + 1, bl], F32, kind="ExternalOutput").ap()

    with tile.TileContext(nc) as tc:
        with tc.tile_pool(name="persist", bufs=1) as pp, \
             tc.tile_pool(name="sp_pe", bufs=2) as sp_pe, \
             tc.tile_pool(name="sp_dve", bufs=2) as sp_dve, \
             tc.tile_pool(name="work", bufs=3) as wp, \
             tc.tile_pool(name="pwsb", bufs=3) as pwp, \
             tc.tile_pool(name="hbuf", bufs=4) as hp, \
             tc.tile_pool(name="h2buf", bufs=4) as h2p, \
             tc.tile_pool(name="ps_h", bufs=2, space="PSUM") as ps_h, \
             tc.tile_pool(name="ps_dz", bufs=2, space="PSUM") as ps_dz, \
             tc.tile_pool(name="ps_t2", bufs=2, space="PSUM") as ps_t2, \
             tc.tile_pool(name="ps_prep", bufs=2, space="PSUM") as ps_prep, \
             tc.tile_pool(name="dram", bufs=1, space="DRAM") as dp:

            # ---- parameter nets (tiny); all small DMAs on gpsimd --------
            par_sb = pp.tile([128, 24], F32, tag="parc")
            nc.gpsimd.dma_start(par_sb[:], par_ap[:])
            t_sb = pp.tile([128, 1], F32, tag="tbc")
            nc.gpsimd.dma_start(t_sb[:], t_ap[:])
            w2t_sb = pp.tile([128, 2048], BF16, tag="w2t")
            nc.gpsimd.dma_start(w2t_sb[:], w2t_ap[:])
            zt_sb = pp.tile([128, bl], BF16, tag="zt")
            nc.sync.dma_start(zt_sb[:], zt_ap[:])
            eye_sb = pp.tile([128, 128], BF16, tag="eye")
            nc.gpsimd.dma_start(eye_sb[:], eye_ap[:])
            b3win_sb = pp.tile([128, fl], F32, tag="b3win")
            b3wout_sb = pp.tile([128, fl], F32, tag="b3wout")
            nc.gpsimd.dma_start(b3win_sb[:], b3win_ap[:])
            nc.gpsimd.dma_start(b3wout_sb[:], b3wout_ap[:])
            b3b_sb = pp.tile([128, nfb], F32, tag="b3b")
            b3gate_sb = pp.tile([128, nfb], F32, tag="b3gate")
            nc.gpsimd.dma_start(b3b_sb[:], b3b_ap[:])
            nc.gpsimd.dma_start(b3gate_sb[:], b3gate_ap[:])

            h0pre = pp.tile([128, 8], F32, tag="h0pre")
            nc.vector.tensor_scalar_mul(h0pre[:], par_sb[:, 0:8], t_sb[:, 0:1])
            nc.vector.tensor_add(h0pre[:], h0pre[:], par_sb[:, 8:16])
            h0_sb = pp.tile([128, 8], BF16, tag="h0")
            nc.scalar.activation(h0_sb[:], h0pre[:],
                                 mybir.ActivationFunctionType.Tanh)

            ps_h1 = ps_prep.tile([128, 8], F32, tag="prep")
            for k4 in range(4):
                for nb in range(2):
                    c = k4 * 2 + nb
                    for mb in range(2):
                        lhs = w2t_sb[:, k4 * 512 + mb * 256 + nb * 128:
                                     k4 * 512 + mb * 256 + nb * 128 + 128]
                        nc.tensor.matmul(ps_h1[:, c:c + 1], lhs,
                                         h0_sb[:, k4 * 2 + mb:k4 * 2 + mb + 1],
                                         start=(mb == 0), stop=(mb == 1))
            h1pre = pp.tile([128, 8], F32, tag="h1pre")
            h1_sb = pp.tile([128, 8], BF16, tag="h1")
            nc.vector.tensor_add(h1pre[:], ps_h1[:], par_sb[:, 16:24])
            nc.scalar.activation(h1_sb[:], h1pre[:],
                                 mybir.ActivationFunctionType.Tanh)
            # h1 -> DRAM in (net, n) order, then broadcast-load nets 0/1
            # replicated across partitions AND repeated dcc times along the
            # free dim (so the DVE multiply runs chunk-granular).
            h1_dram = dp.tile([8, 128], BF16, tag="h1d")
            nc.gpsimd.dma_start(h1_dram.rearrange("c n -> n c"), h1_sb[:])
            h1b = []
            for k4 in range(2):
                hb = pp.tile([128, dcc * N], BF16, tag=f"h1b{k4}")
                src = h1_dram.rearrange("c n -> (c n)")[k4 * N:(k4 + 1) * N]
                src = src.unsqueeze(0).unsqueeze(0)
                nc.gpsimd.dma_start(hb[:], src.broadcast_to([128, dcc, N]))
                h1b.append(hb)

            # heads: b and gate (psum [f, fb] columns), early -- gate gates
            # the per-fb transposes below.
            b_sb = pp.tile([128, nfb], F32, tag="bh")
            gate_sb = pp.tile([128, nfb], F32, tag="gate")
            gpre = pp.tile([128, nfb], F32, tag="gpre")
            for w3hT_ap, bias_sb, dst, net in ((w3bT_ap, b3b_sb, b_sb, 2),
                                               (w3gateT_ap, b3gate_sb, gpre, 3)):
                w3ht = pp.tile([128, 2 * fl], BF16, tag=f"w3head{net}")
                nc.gpsimd.dma_start(
                    w3ht[:], w3hT_ap.rearrange("(nb p) fl -> p nb fl", p=128))
                phd = ps_prep.tile([128, nfb], F32, tag="prep")
                for a in range(nfb):
                    for nb in range(2):
                        nc.tensor.matmul(
                            phd[:, a:a + 1],
                            w3ht[:, nb * fl + a * 128:nb * fl + (a + 1) * 128],
                            h1_sb[:, net * 2 + nb:net * 2 + nb + 1],
                            start=(nb == 0), stop=(nb == 1))
                nc.vector.tensor_add(dst[:], phd[:], bias_sb[:])
            nc.scalar.activation(gate_sb[:], gpre[:],
                                 mybir.ActivationFunctionType.Sigmoid)
            b_bf = pp.tile([128, nfb], BF16, tag="bbf")
            nc.vector.tensor_copy(b_bf[:], b_sb[:])

            # ---- phase 1: sharded matvecs, split across PE and DVE ------
            # All stream DMAs ride the sync HWDGE ring, in emission order.
            # Vector runs ONLY the DVE path; PE-path drains go through
            # scalar (psum->sbuf copy) + gpsimd (bias add).
            w_inT_bf = pp.tile([128, fl], BF16, tag="winT")
            w_outT_bf = pp.tile([128, fl], BF16, tag="woutT")
            daccs = {}
            for net, nm in ((0, "win"), (1, "wout")):
                daccs[net] = pp.tile([128, max(dve_cols, 1)], F32,
                                     tag=f"dacc{net}", name=f"dacc{nm}")

            def emit_pe_chunk(w3T_ap, bias_sb, dst, net, c):
                # one big DMA carries both n-halves of this chunk
                w3t = sp_pe.tile([128, 2 * chunk_r], BF16, tag="w3chunk")
                nc.sync.dma_start(
                    w3t[:], w3T_ap[:, c * 2 * chunk_r:(c + 1) * 2 * chunk_r])
                pw = ps_prep.tile([128, rpc], F32, tag="prep")
                for a in range(rpc):
                    for nb in range(2):
                        nc.tensor.matmul(
                            pw[:, a:a + 1],
                            w3t[:, nb * chunk_r + a * 128:
                                nb * chunk_r + (a + 1) * 128],
                            h1_sb[:, net * 2 + nb:net * 2 + nb + 1],
                            start=(nb == 0), stop=(nb == 1))
                pw_sb = pwp.tile([128, rpc], F32, tag="pwsb")
                nc.scalar.copy(pw_sb[:], pw[:])
                nc.gpsimd.tensor_add(dst[:, c * rpc:(c + 1) * rpc], pw_sb[:],
                                     bias_sb[:, c * rpc:(c + 1) * rpc])

            def emit_dve_pair(w3N_ap, bias_sb, dst, net, cp):
                # one [128, 2*dcc*N] DMA = two dcc-column compute chunks
                w3n = sp_dve.tile([128, 2 * dcc * N], BF16, tag="w3nat")
                nc.scalar.dma_start(w3n[:], w3N_ap[cp * 128:(cp + 1) * 128, :])
                hn = N // 2
                for s in range(2):
                    c = cp * 2 + s
                    prod = wp.tile([128, dcc * N], BF16, tag="prod")
                    nc.vector.tensor_mul(prod[:],
                                         w3n[:, s * dcc * N:(s + 1) * dcc * N],
                                         h1b[net][:])
                    # fold n-halves within each f-column (bf16, 2x rate),
                    # then reduce the half-size rest
                    pv = prod.rearrange("p (a n) -> p a n", a=dcc)
                    fold = wp.tile([128, dcc * hn], BF16, tag="fold")
                    fv = fold.rearrange("p (a n) -> p a n", a=dcc)
                    nc.vector.tensor_add(fv, pv[:, :, 0:hn], pv[:, :, hn:N])
                    nc.vector.tensor_reduce(
                        daccs[net][:, c * dcc:(c + 1) * dcc],
                        fv, mybir.AxisListType.X, ADD)
                    if c == n_dve_chunks - 1:
                        nc.vector.tensor_add(dst[:, pe_cols:fl],
                                             daccs[net][:, 0:dve_cols],
                                             bias_sb[:, pe_cols:fl])

            blob_in = dp.tile([1, BLOB], BF16, tag="blobi", name="blobi")
            blob_out = dp.tile([n_cores, BLOB], BF16, tag="blobo", name="blobo",
                               addr_space="Shared")
            w_outg = pp.tile([128, nfb * 128], BF16, tag="woutg")
            w_in_fz = pp.tile([128, nfb * 128], BF16, tag="winfz")
            sg = pp.tile([128, nfb], F32, tag="sg")
            sg_bf = pp.tile([128, nfb], BF16, tag="sgbf")

            def emit_fb_group(fb):
                # transpose w_in/w_out block to [f, z]; fold gate into w_out;
                # sg; then stream this block's slice of the blob out.
                ptr = ps_prep.tile([128, 128], BF16, tag="prep")
                nc.tensor.transpose(ptr[:], w_outT_bf[:, fb * 128:(fb + 1) * 128],
                                    eye_sb[:])
                nc.vector.tensor_scalar_mul(w_outg[:, fb * 128:(fb + 1) * 128],
                                            ptr[:], gate_sb[:, fb:fb + 1])
                pti = ps_prep.tile([128, 128], BF16, tag="prep")
                nc.tensor.transpose(pti[:], w_inT_bf[:, fb * 128:(fb + 1) * 128],
                                    eye_sb[:])
                nc.vector.tensor_copy(w_in_fz[:, fb * 128:(fb + 1) * 128], pti[:])
                # sg = sum_z w_in[f,z] * w_out[f,z] * gate[f]
                prod = wp.tile([128, 128], F32, tag="sprod")
                nc.vector.tensor_mul(prod[:], w_in_fz[:, fb * 128:(fb + 1) * 128],
                                     w_outg[:, fb * 128:(fb + 1) * 128])
                nc.vector.tensor_reduce(sg[:, fb:fb + 1], prod[:],
                                        mybir.AxisListType.X, ADD)
                nc.sync.dma_start(
                    blob_in[0, 0:SZ_A]
                    .rearrange("(z f) -> z f", z=128)[:, fb * 128:(fb + 1) * 128],
                    w_inT_bf[:, fb * 128:(fb + 1) * 128])
                nc.sync.dma_start(
                    blob_in[0, OF_B + fb * 128 * 128:OF_B + (fb + 1) * 128 * 128]
                    .rearrange("(f zz) -> f zz", f=128),
                    w_outg[:, fb * 128:(fb + 1) * 128])

            # interleave PE/DVE chunks; hold the last DVE pairs until after
            # the fb0 group so fb0's blob slice streams out early.
            pe_units = []
            dve_units = []
            for c in range(n_pe_chunks):
                pe_units.append((w3winT_ap, b3win_sb, w_inT_bf, 0, c))
                pe_units.append((w3woutT_ap, b3wout_sb, w_outT_bf, 1, c))
            n_dve_pairs = n_dve_chunks // 2
            for cp in range(n_dve_pairs):
                dve_units.append((w3winN_ap, b3win_sb, w_inT_bf, 0, cp))
                dve_units.append((w3woutN_ap, b3wout_sb, w_outT_bf, 1, cp))
            npe, ndve = len(pe_units), len(dve_units)
            dve_hold = min(2, ndve)
            di = 0
            for pi in range(npe):
                emit_pe_chunk(*pe_units[pi])
                dt = min((pi + 1) * ndve // npe, ndve - dve_hold)
                while di < dt:
                    emit_dve_pair(*dve_units[di])
                    di += 1
            emit_fb_group(0)          # fb0 = cols [0,128) -- PE part complete
            while di < ndve:
                emit_dve_pair(*dve_units[di])
                di += 1
            emit_fb_group(1)          # fb1 = cols [128,256)
            nc.vector.tensor_copy(sg_bf[:], sg[:])
            nc.sync.dma_start(
                blob_in[0, OF_C:OF_C + SZ_C].rearrange("(fb f) -> f fb", fb=nfb),
                sg_bf[:])
            nc.sync.dma_start(
                blob_in[0, OF_D:OF_D + SZ_D].rearrange("(fb f) -> f fb", fb=nfb),
                b_bf[:])
            nc.gpsimd.collective_compute(
                "AllGather", mybir.AluOpType.bypass,
                replica_groups=[list(range(n_cores))],
                ins=[blob_in.opt()], outs=[blob_out.opt()])

            # keep the PE's HAM clock warm across the collective window:
            # a serial junk chain (matmul -> vector copy -> matmul ...)
            # paced at ~1.5us per link, ending roughly when the AG lands.
            wch = pp.tile([128, 128], BF16, tag="wch")
            nc.vector.tensor_copy(wch[:], eye_sb[:])
            for _ in range(26):
                pj = ps_prep.tile([128, 128], F32, tag="prep")
                nc.tensor.matmul(pj[:], eye_sb[:], wch[:],
                                 start=True, stop=True)
                wch = pwp.tile([128, 128], BF16, tag="wchx")
                nc.vector.tensor_copy(wch[:], pj[:])

            # ---- post-AG loads: global stationary tiles, per f-block ----
            sg_g = pp.tile([128, nfb_g], BF16, tag="sgg")
            b_gbf = pp.tile([128, nfb_g], BF16, tag="bgbf")
            for fb in range(nfb):
                nc.gpsimd.dma_start(
                    sg_g.rearrange("ff (r fb) -> ff r fb",
                                   r=n_cores)[:, :, fb],
                    blob_out[:, OF_C + fb * 128:OF_C + (fb + 1) * 128]
                    .rearrange("r ff -> ff r"))
                nc.gpsimd.dma_start(
                    b_gbf.rearrange("ff (r fb) -> ff r fb",
                                    r=n_cores)[:, :, fb],
                    blob_out[:, OF_D + fb * 128:OF_D + (fb + 1) * 128]
                    .rearrange("r ff -> ff r"))
            b_g = pp.tile([128, nfb_g], F32, tag="bg")
            nc.vector.tensor_copy(b_g[:], b_gbf[:])

            # global fb index g = r*nfb + fb
            w_inT_g = pp.tile([128, f], BF16, tag="winTg")
            w_outg_g = pp.tile([128, f], BF16, tag="woutgg")
            w_inT_gv = w_inT_g.rearrange("z (r fb ff) -> z r fb ff",
                                         r=n_cores, fb=nfb)
            w_outg_gv = w_outg_g.rearrange("ff (r fb zz) -> ff r fb zz",
                                           r=n_cores, fb=nfb)
            for fb in range(nfb):
                nc.sync.dma_start(
                    w_inT_gv[:, :, fb, :],
                    blob_out[:, 0:SZ_A]
                    .rearrange("r (z fb ff) -> z r fb ff",
                               z=128, fb=nfb)[:, :, fb, :])
                nc.scalar.dma_start(
                    w_outg_gv[:, :, fb, :],
                    blob_out[:, OF_B + fb * 128 * 128:
                             OF_B + (fb + 1) * 128 * 128]
                    .rearrange("r (ff zz) -> ff r zz", ff=128))

            # cneg = -sum_f sg / F  (global)
            sgs = pp.tile([128, 1], F32, tag="sgs")
            nc.vector.tensor_reduce(sgs[:], sg_g[:], mybir.AxisListType.X, ADD)
            csum = pp.tile([1, 1], F32, tag="csum")
            nc.gpsimd.tensor_reduce(csum[:], sgs[:], mybir.AxisListType.XYZWC,
                                    ADD)
            cneg = pp.tile([1, 1], F32, tag="cneg")
            nc.scalar.mul(cneg[:], csum[:], -1.0 / f)

            # ---- stage B: own batch shard x full F ----------------------
            for j in range(nbc):
                b0 = j * bc
                pdz = ps_dz.tile([128, bc], F32, tag="pdz")
                pt2 = ps_t2.tile([1, bc], F32, tag="pt2")
                hs = [None] * nfb_g
                h2s = [None] * nfb_g

                def emit_ph(fb):
                    ph = ps_h.tile([128, bc], F32, tag="ph")
                    nc.tensor.matmul(ph[:],
                                     w_inT_g[:, fb * 128:(fb + 1) * 128],
                                     zt_sb[:, b0:b0 + bc],
                                     start=True, stop=True)
                    h_bf = hp.tile([128, bc], BF16, tag="hbf")
                    nc.scalar.activation(h_bf[:], ph[:],
                                         mybir.ActivationFunctionType.Tanh,
                                         bias=b_g[:, fb:fb + 1])
                    h2_bf = h2p.tile([128, bc], BF16, tag="h2bf")
                    nc.vector.tensor_mul(h2_bf[:], h_bf[:], h_bf[:])
                    hs[fb] = h_bf
                    h2s[fb] = h2_bf

                def emit_acc(fb):
                    nc.tensor.matmul(pdz[:],
                                     w_outg_g[:, fb * 128:(fb + 1) * 128],
                                     hs[fb][:],
                                     start=(fb == 0), stop=(fb == nfb_g - 1))
                    nc.tensor.matmul(pt2[:], sg_g[:, fb:fb + 1], h2s[fb][:],
                                     start=(fb == 0), stop=(fb == nfb_g - 1))

                for fb in range(nfb_g):
                    emit_ph(fb)
                    if fb >= 1:
                        emit_acc(fb - 1)
                emit_acc(nfb_g - 1)

                dz_sb = wp.tile([128, bc], F32, tag="dzsb")
                nc.scalar.mul(dz_sb[:], pdz[:], 1.0 / f)
                nc.sync.dma_start(out_ap[0:Z, b0:b0 + bc], dz_sb[:])
                tr_sb = wp.tile([1, bc], F32, tag="trsb")
                nc.scalar.activation(
                    tr_sb[:], pt2[:],
                    mybir.ActivationFunctionType.Identity,
                    bias=cneg[0:1, 0:1], scale=1.0 / f)
                nc.gpsimd.dma_start(out_ap[Z:Z + 1, b0:b0 + bc], tr_sb[:])

    nc.compile()
    return nc


def host_prep(t, z_and_logpz, W1, B1, W2, B2, W3_win, b3_win,
              W3_wout, b3_wout, W3_b, b3_b, W3_gate, b3_gate,
              n_cores=N_CORES, b=B, f=F, pe_cols=PE_COLS):
    """Shard + lay out the numpy inputs into per-core in_maps."""
    fl = f // n_cores
    nfb = fl // 128
    rows = fl * Z
    rows_pe = pe_cols * 128
    n_pe_chunks = rows_pe // CHUNK_R

    dcc2 = 2 * DVE_CC   # f-columns per DVE DMA pair

    def pack_pe(x):  # [rows_pe, N] -> [128, nch*2*chunk_r], chunk-major
        a = np.ascontiguousarray(x.T)                 # [256, rows_pe]
        return np.ascontiguousarray(
            a.reshape(2, 128, n_pe_chunks, CHUNK_R).transpose(1, 2, 0, 3)
            .reshape(128, n_pe_chunks * 2 * CHUNK_R))

    def pack_nat(x):  # [rows_dve, N] -> [nch*128, dcc2*N], partition-contig
        nch = x.shape[0] // (dcc2 * 128)
        return np.ascontiguousarray(
            x.reshape(nch, dcc2, 128, N).transpose(0, 2, 1, 3)
            .reshape(nch * 128, dcc2 * N))

    def col8(x):  # [4, 256] -> [128, 8] with col = k*2 + nb
        return np.ascontiguousarray(
            np.asarray(x, np.float32).reshape(4, 2, 128).transpose(2, 0, 1)
            .reshape(128, 8))

    t_in = np.ascontiguousarray(
        np.broadcast_to(np.asarray(t, np.float32).reshape(1, 1), (128, 1)))
    parc = np.ascontiguousarray(np.concatenate(
        [col8(np.asarray(W1, np.float32)[:, :, 0]), col8(B1), col8(B2)],
        axis=1))
    # lhsT tile for h1 net: [m128, (k4, mb, n)] = W2[k4, n, mb*128+m128]
    w2tc = np.ascontiguousarray(
        np.asarray(W2, np.float32).transpose(0, 2, 1)        # [k, m, n]
        .reshape(4, 2, 128, 256).transpose(2, 0, 1, 3).reshape(128, 2048)).astype(BF)
    w3win_bf = np.asarray(W3_win, np.float32).astype(BF)
    w3wout_bf = np.asarray(W3_wout, np.float32).astype(BF)
    w3b_bf = np.asarray(W3_b, np.float32).astype(BF)
    w3gate_bf = np.asarray(W3_gate, np.float32).astype(BF)
    b3win = np.asarray(b3_win, np.float32)
    b3wout = np.asarray(b3_wout, np.float32)
    b3b = np.asarray(b3_b, np.float32)
    b3gate = np.asarray(b3_gate, np.float32)
    z = np.asarray(z_and_logpz, np.float32)[:, :Z]
    ztb = np.ascontiguousarray(z.T).astype(BF)
    eye = np.eye(128, dtype=np.float32).astype(BF)
    bl = b // n_cores

    in_maps = []
    for k in range(n_cores):
        r0 = k * rows
        f0 = k * fl
        in_maps.append({
            "t": t_in, "parc": parc, "w2tc": w2tc,
            "w3winT_sl": pack_pe(w3win_bf[r0:r0 + rows_pe]),
            "w3woutT_sl": pack_pe(w3wout_bf[r0:r0 + rows_pe]),
            "w3winN_sl": pack_nat(w3win_bf[r0 + rows_pe:r0 + rows]),
            "w3woutN_sl": pack_nat(w3wout_bf[r0 + rows_pe:r0 + rows]),
            "b3win_c": np.ascontiguousarray(
                b3win[r0:r0 + rows].reshape(fl, 128).T),
            "b3wout_c": np.ascontiguousarray(
                b3wout[r0:r0 + rows].reshape(fl, 128).T),
            "w3bT_sl": np.ascontiguousarray(w3b_bf[f0:f0 + fl].T),
            "w3gateT_sl": np.ascontiguousarray(w3gate_bf[f0:f0 + fl].T),
            "b3b_c": np.ascontiguousarray(b3b[f0:f0 + fl].reshape(nfb, 128).T),
            "b3gate_c": np.ascontiguousarray(
                b3gate[f0:f0 + fl].reshape(nfb, 128).T),
            "ztb_sl": np.ascontiguousarray(ztb[:, k * bl:(k + 1) * bl]),
            "eyeb": eye,
        })
    return in_maps


_NC_CACHE = {}


def kernel(**inputs) -> np.ndarray:
    _ensure_ntff_hook()
    from concourse import bass_utils

    key = "full"
    if key not in _NC_CACHE:
        _NC_CACHE[key] = build_module()
    nc = _NC_CACHE[key]

    in_maps = host_prep(**inputs)
    res = bass_utils.run_bass_kernel_spmd(nc, in_maps, list(range(N_CORES)))
    bl = B // N_CORES
    out = np.empty((B, Z + 1), np.float32)
    for k in range(N_CORES):
        out[k * bl:(k + 1) * bl, :] = res.results[k]["out"].T
    return out


# revision 23
# speedup vs baseline: 1.0195x; 1.0195x over previous
"""Trainium2 Bass kernel for nn_ContinousNormalizingFlowRHS.

Computes, for z in R^{B x Z} and scalar time t:
  h0 = tanh(W1*t + B1); h1 = tanh(einsum('knm,km->kn', W2, h0) + B2)
  w_in  = (W3_win  @ h1[0] + b3_win ).reshape(F, Z)
  w_out = (W3_wout @ h1[1] + b3_wout).reshape(F, Z)
  b     =  W3_b    @ h1[2] + b3_b
  gate  = sigmoid(W3_gate @ h1[3] + b3_gate)
  h = tanh(z @ w_in.T + b); dz = (h*gate) @ w_out / F
  trace = ((1-h^2)*gate) @ (sum(w_in*w_out,1)) / F
  out = concat([dz, -trace[:,None]], -1)

Strategy (8 NeuronCores, single SPMD launch):
  Phase 1 (f-sharded): stream W3_win/W3_wout (33.6 MB/core bf16) on the
  sync HWDGE ring only; consume with the PE (stationary-side matvec,
  FWL) for pe_cols f-columns per matrix and the DVE (mul + fold + reduce)
  for the rest.  Engine queues are kept path-pure: Vector only runs the
  DVE path (PE-path drains go scalar-copy -> gpsimd bias-add), so no
  cross-path head-of-line blocking.
  Each core packs its local (w_inT, gate*w_out, sg, b) into a ~129 KB
  blob; one AllGather replicates all slices.
  Stage B (batch-sharded): each core runs its OWN B/8 batch shard
  against the FULL F, accumulating dz / trace in fp32 PSUM across all
  16 f-blocks, and writes its final [Z+1, B/8] output directly.
"""

import sys
import types
import numpy as np
import ml_dtypes

BF = ml_dtypes.bfloat16

# problem sizes (hardcoded per contract)
Z = 128
N = 256
F = 2048
B = 8192
N_CORES = 8

PE_COLS = 160       # per matrix: local f-columns computed on the PE (rest DVE)
CHUNK_R = 5120      # W3 rows per streamed PE chunk ([128, 5120] bf16 tiles)
DVE_CC = 16         # f-columns per DVE chunk (2048 rows)
BC = 512            # batch columns per stage-B chunk (one PSUM bank)


def _ensure_ntff_hook():
    """run_bass_kernel_spmd(trace=True) under axon needs antenv.axon_hooks."""
    if 'antenv.axon_hooks' in sys.modules:
        return
    try:
        from trn_agent_boot.trn_boot import _ntff_profile_via_ctypes
        hook = _ntff_profile_via_ctypes('/opt/axon/libaxon_pjrt.so')
    except Exception:
        hook = None
    try:
        import antenv
    except Exception:
        return
    mod = types.ModuleType('antenv.axon_hooks')
    mod.get_axon_ntff_profile_hook = lambda: hook
    mod.set_axon_ntff_profile_hook = lambda h: None
    sys.modules['antenv.axon_hooks'] = mod
    antenv.axon_hooks = mod


def build_module(n_cores=N_CORES, b=B, f=F, pe_cols=PE_COLS, chunk_r=CHUNK_R,
                 bc=BC, debug=False):
    """Build the Bass module (SPMD program, one per core)."""
    import concourse.tile as tile
    from concourse import bacc, mybir

    F32 = mybir.dt.float32
    BF16 = mybir.dt.bfloat16
    ADD = mybir.AluOpType.add

    fl = f // n_cores            # local f count
    nfb = fl // 128              # local f blocks of 128
    nfb_g = f // 128             # global f blocks of 128
    rows_pe = pe_cols * 128      # rows of W3 handled by the PE
    dve_cols = fl - pe_cols
    rows_dve = dve_cols * 128
    n_pe_chunks = rows_pe // chunk_r
    rpc = chunk_r // 128         # w columns produced per PE chunk
    dcc = DVE_CC                 # f-columns per DVE chunk
    n_dve_chunks = dve_cols // dcc
    bl = b // n_cores            # per-core batch shard
    nbc = bl // bc               # stage-B batch chunks
    assert rows_pe % chunk_r == 0 and dve_cols % dcc == 0

    # blob layout (bf16 elements): w_inT (z,f) | w_outg (fb,f,z) | sg | b
    SZ_A = 128 * fl
    SZ_B = 128 * fl
    SZ_C = fl
    SZ_D = fl
    BLOB = SZ_A + SZ_B + SZ_C + SZ_D
    OF_B, OF_C, OF_D = SZ_A, SZ_A + SZ_B, SZ_A + SZ_B + SZ_C

    nc = bacc.Bacc("TRN2", target_bir_lowering=False, debug=debug,
                   num_devices=n_cores)

    def inp(name, shape, dt):
        return nc.dram_tensor(name, shape, dt, kind="ExternalInput").ap()

    t_ap = inp("t", [128, 1], F32)                  # t replicated
    par_ap = inp("parc", [128, 24], F32)            # w1c | b1c | b2c
    w2t_ap = inp("w2tc", [128, 2048], BF16)
    # PE slabs: one [128, 2*chunk_r] DMA per chunk (both n-halves)
    w3winT_ap = inp("w3winT_sl", [128, 2 * rows_pe], BF16)
    w3woutT_ap = inp("w3woutT_sl", [128, 2 * rows_pe], BF16)
    # DVE slabs: one [128, 2*dcc*N] DMA per pair of compute chunks
    w3winN_ap = inp("w3winN_sl", [rows_dve // (2 * dcc * 128) * 128,
                                  2 * dcc * N], BF16)
    w3woutN_ap = inp("w3woutN_sl", [rows_dve // (2 * dcc * 128) * 128,
                                    2 * dcc * N], BF16)
    b3win_ap = inp("b3win_c", [128, fl], F32)
    b3wout_ap = inp("b3wout_c", [128, fl], F32)
    w3bT_ap = inp("w3bT_sl", [N, fl], BF16)
    w3gateT_ap = inp("w3gateT_sl", [N, fl], BF16)
    b3b_ap = inp("b3b_c", [128, nfb], F32)
    b3gate_ap = inp("b3gate_c", [128, nfb], F32)
    zt_ap = inp("ztb_sl", [128, bl], BF16)          # own batch shard only
    eye_ap = inp("eyeb", [128, 128], BF16)
    out_ap = nc.dram_tensor("out", [Z + 1, bl], F32, kind="ExternalOutput").ap()

    with tile.TileContext(nc) as tc:
        with tc.tile_pool(name="persist", bufs=1) as pp, \
             tc.tile_pool(name="sp_pe", bufs=2) as sp_pe, \
             tc.tile_pool(name="sp_dve", bufs=2) as sp_dve, \
             tc.tile_pool(name="work", bufs=3) as wp, \
             tc.tile_pool(name="pwsb", bufs=3) as pwp, \
             tc.tile_pool(name="hbuf", bufs=4) as hp, \
             tc.tile_pool(name="h2buf", bufs=4) as h2p, \
             tc.tile_pool(name="ps_h", bufs=2, space="PSUM") as ps_h, \
             tc.tile_pool(name="ps_dz", bufs=2, space="PSUM") as ps_dz, \
             tc.tile_pool(name="ps_t2", bufs=2, space="PSUM") as ps_t2, \
             tc.tile_pool(name="ps_prep", bufs=2, space="PSUM") as ps_prep, \
             tc.tile_pool(name="dram", bufs=1, space="DRAM") as dp:

            # ---- parameter nets (tiny); all small DMAs on gpsimd --------
            par_sb = pp.tile([128, 24], F32, tag="parc")
            nc.gpsimd.dma_start(par_sb[:], par_ap[:])
            t_sb = pp.tile([128, 1], F32, tag="tbc")
            nc.gpsimd.dma_start(t_sb[:], t_ap[:])
            w2t_sb = pp.tile([128, 2048], BF16, tag="w2t")
            nc.gpsimd.dma_start(w2t_sb[:], w2t_ap[:])
            zt_sb = pp.tile([128, bl], BF16, tag="zt")
            nc.sync.dma_start(zt_sb[:], zt_ap[:])
            eye_sb = pp.tile([128, 128], BF16, tag="eye")
            nc.gpsimd.dma_start(eye_sb[:], eye_ap[:])
            b3win_sb = pp.tile([128, fl], F32, tag="b3win")
            b3wout_sb = pp.tile([128, fl], F32, tag="b3wout")
            nc.gpsimd.dma_start(b3win_sb[:], b3win_ap[:])
            nc.gpsimd.dma_start(b3wout_sb[:], b3wout_ap[:])
            b3b_sb = pp.tile([128, nfb], F32, tag="b3b")
            b3gate_sb = pp.tile([128, nfb], F32, tag="b3gate")
            nc.gpsimd.dma_start(b3b_sb[:], b3b_ap[:])
            nc.gpsimd.dma_start(b3gate_sb[:], b3gate_ap[:])

            h0pre = pp.tile([128, 8], F32, tag="h0pre")
            nc.vector.tensor_scalar_mul(h0pre[:], par_sb[:, 0:8], t_sb[:, 0:1])
            nc.vector.tensor_add(h0pre[:], h0pre[:], par_sb[:, 8:16])
            h0_sb = pp.tile([128, 8], BF16, tag="h0")
            nc.scalar.activation(h0_sb[:], h0pre[:],
                                 mybir.ActivationFunctionType.Tanh)

            ps_h1 = ps_prep.tile([128, 8], F32, tag="prep")
            for k4 in range(4):
                for nb in range(2):
                    c = k4 * 2 + nb
                    for mb in range(2):
                        lhs = w2t_sb[:, k4 * 512 + mb * 256 + nb * 128:
                                     k4 * 512 + mb * 256 + nb * 128 + 128]
                        nc.tensor.matmul(ps_h1[:, c:c + 1], lhs,
                                         h0_sb[:, k4 * 2 + mb:k4 * 2 + mb + 1],
                                         start=(mb == 0), stop=(mb == 1))
            h1pre = pp.tile([128, 8], F32, tag="h1pre")
            h1_sb = pp.tile([128, 8], BF16, tag="h1")
            nc.vector.tensor_add(h1pre[:], ps_h1[:], par_sb[:, 16:24])
            nc.scalar.activation(h1_sb[:], h1pre[:],
                                 mybir.ActivationFunctionType.Tanh)
            # h1 -> DRAM in (net, n) order, then broadcast-load nets 0/1
            # replicated across partitions AND repeated dcc times along the
            # free dim (so the DVE multiply runs chunk-granular).
            h1_dram = dp.tile([8, 128], BF16, tag="h1d")
            nc.gpsimd.dma_start(h1_dram.rearrange("c n -> n c"), h1_sb[:])
            h1b = []
            for k4 in range(2):
                hb = pp.tile([128, dcc * N], BF16, tag=f"h1b{k4}")
                src = h1_dram.rearrange("c n -> (c n)")[k4 * N:(k4 + 1) * N]
                src = src.unsqueeze(0).unsqueeze(0)
                nc.gpsimd.dma_start(hb[:], src.broadcast_to([128, dcc, N]))
                h1b.append(hb)

            # heads: b and gate (psum [f, fb] columns), early -- gate gates
            # the per-fb transposes below.
            b_sb = pp.tile([128, nfb], F32, tag="bh")
            gate_sb = pp.tile([128, nfb], F32, tag="gate")
            gpre = pp.tile([128, nfb], F32, tag="gpre")
            for w3hT_ap, bias_sb, dst, net in ((w3bT_ap, b3b_sb, b_sb, 2),
                                               (w3gateT_ap, b3gate_sb, gpre, 3)):
                w3ht = pp.tile([128, 2 * fl], BF16, tag=f"w3head{net}")
                nc.gpsimd.dma_start(
                    w3ht[:], w3hT_ap.rearrange("(nb p) fl -> p nb fl", p=128))
                phd = ps_prep.tile([128, nfb], F32, tag="prep")
                for a in range(nfb):
                    for nb in range(2):
                        nc.tensor.matmul(
                            phd[:, a:a + 1],
                            w3ht[:, nb * fl + a * 128:nb * fl + (a + 1) * 128],
                            h1_sb[:, net * 2 + nb:net * 2 + nb + 1],
                            start=(nb == 0), stop=(nb == 1))
                nc.vector.tensor_add(dst[:], phd[:], bias_sb[:])
            nc.scalar.activation(gate_sb[:], gpre[:],
                                 mybir.ActivationFunctionType.Sigmoid)
            b_bf = pp.tile([128, nfb], BF16, tag="bbf")
            nc.vector.tensor_copy(b_bf[:], b_sb[:])

            # ---- phase 1: sharded matvecs, split across PE and DVE ------
            # All stream DMAs ride the sync HWDGE ring, in emission order.
            # Vector runs ONLY the DVE path; PE-path drains go through
            # scalar (psum->sbuf copy) + gpsimd (bias add).
            w_inT_bf = pp.tile([128, fl], BF16, tag="winT")
            w_outT_bf = pp.tile([128, fl], BF16, tag="woutT")
            daccs = {}
            for net, nm in ((0, "win"), (1, "wout")):
                daccs[net] = pp.tile([128, max(dve_cols, 1)], F32,
                                     tag=f"dacc{net}", name=f"dacc{nm}")

            def emit_pe_chunk(w3T_ap, bias_sb, dst, net, c, ring):
                # one big DMA carries both n-halves of this chunk
                w3t = sp_pe.tile([128, 2 * chunk_r], BF16, tag="w3chunk")
                ring.dma_start(
                    w3t[:], w3T_ap[:, c * 2 * chunk_r:(c + 1) * 2 * chunk_r])
                pw = ps_prep.tile([128, rpc], F32, tag="prep")
                for a in range(rpc):
                    for nb in range(2):
                        nc.tensor.matmul(
                            pw[:, a:a + 1],
                            w3t[:, nb * chunk_r + a * 128:
                                nb * chunk_r + (a + 1) * 128],
                            h1_sb[:, net * 2 + nb:net * 2 + nb + 1],
                            start=(nb == 0), stop=(nb == 1))
                nc.vector.tensor_add(dst[:, c * rpc:(c + 1) * rpc], pw[:],
                                     bias_sb[:, c * rpc:(c + 1) * rpc])

            def emit_dve_trigger(w3N_ap, cp):
                # one [128, 2*dcc*N] DMA = two dcc-column compute chunks
                w3n = sp_dve.tile([128, 2 * dcc * N], BF16, tag="w3nat")
                nc.scalar.dma_start(w3n[:], w3N_ap[cp * 128:(cp + 1) * 128, :])
                return w3n

            def emit_dve_compute(w3n, bias_sb, dst, net, cp):
                hn = N // 2
                for s in range(2):
                    c = cp * 2 + s
                    prod = wp.tile([128, dcc * N], BF16, tag="prod")
                    nc.vector.tensor_mul(prod[:],
                                         w3n[:, s * dcc * N:(s + 1) * dcc * N],
                                         h1b[net][:])
                    # fold n-halves within each f-column (bf16, 2x rate),
                    # then reduce the half-size rest
                    pv = prod.rearrange("p (a n) -> p a n", a=dcc)
                    fold = wp.tile([128, dcc * hn], BF16, tag="fold")
                    fv = fold.rearrange("p (a n) -> p a n", a=dcc)
                    nc.vector.tensor_add(fv, pv[:, :, 0:hn], pv[:, :, hn:N])
                    nc.vector.tensor_reduce(
                        daccs[net][:, c * dcc:(c + 1) * dcc],
                        fv, mybir.AxisListType.X, ADD)
                    if c == n_dve_chunks - 1:
                        nc.vector.tensor_add(dst[:, pe_cols:fl],
                                             daccs[net][:, 0:dve_cols],
                                             bias_sb[:, pe_cols:fl])

            blob_in = dp.tile([1, BLOB], BF16, tag="blobi", name="blobi")
            blob_out = dp.tile([n_cores, BLOB], BF16, tag="blobo", name="blobo",
                               addr_space="Shared")
            w_outg = pp.tile([128, nfb * 128], BF16, tag="woutg")
            w_in_fz = pp.tile([128, nfb * 128], BF16, tag="winfz")
            sg = pp.tile([128, nfb], F32, tag="sg")
            sg_bf = pp.tile([128, nfb], BF16, tag="sgbf")

            def emit_fb_group(fb):
                # transpose w_in/w_out block to [f, z]; fold gate into w_out;
                # sg; then stream this block's slice of the blob out.
                ptr = ps_prep.tile([128, 128], BF16, tag="prep")
                nc.tensor.transpose(ptr[:], w_outT_bf[:, fb * 128:(fb + 1) * 128],
                                    eye_sb[:])
                nc.vector.tensor_scalar_mul(w_outg[:, fb * 128:(fb + 1) * 128],
                                            ptr[:], gate_sb[:, fb:fb + 1])
                pti = ps_prep.tile([128, 128], BF16, tag="prep")
                nc.tensor.transpose(pti[:], w_inT_bf[:, fb * 128:(fb + 1) * 128],
                                    eye_sb[:])
                nc.vector.tensor_copy(w_in_fz[:, fb * 128:(fb + 1) * 128], pti[:])
                # sg = sum_z w_in[f,z] * w_out[f,z] * gate[f]
                prod = wp.tile([128, 128], F32, tag="sprod")
                nc.vector.tensor_mul(prod[:], w_in_fz[:, fb * 128:(fb + 1) * 128],
                                     w_outg[:, fb * 128:(fb + 1) * 128])
                nc.vector.tensor_reduce(sg[:, fb:fb + 1], prod[:],
                                        mybir.AxisListType.X, ADD)
                nc.sync.dma_start(
                    blob_in[0, 0:SZ_A]
                    .rearrange("(z f) -> z f", z=128)[:, fb * 128:(fb + 1) * 128],
                    w_inT_bf[:, fb * 128:(fb + 1) * 128])
                nc.sync.dma_start(
                    blob_in[0, OF_B + fb * 128 * 128:OF_B + (fb + 1) * 128 * 128]
                    .rearrange("(f zz) -> f zz", f=128),
                    w_outg[:, fb * 128:(fb + 1) * 128])

            # DVE-slab triggers all up-front on the (otherwise idle) scalar
            # ring, paced only by sp_dve buffer recycling; PE slabs split
            # across the sync and gpsimd rings.  Vector interleaves DVE
            # computes with PE-chunk drains at byte-proportional positions.
            n_dve_pairs = n_dve_chunks // 2
            dve_units = []
            for cp in range(n_dve_pairs):
                dve_units.append((w3winN_ap, b3win_sb, w_inT_bf, 0, cp))
                dve_units.append((w3woutN_ap, b3wout_sb, w_outT_bf, 1, cp))
            dve_tiles = [emit_dve_trigger(u[0], u[4]) for u in dve_units]
            pe_units = []
            for c in range(n_pe_chunks):
                pe_units.append((w3winT_ap, b3win_sb, w_inT_bf, 0, c))
                pe_units.append((w3woutT_ap, b3wout_sb, w_outT_bf, 1, c))
            npe, ndve = len(pe_units), len(dve_units)
            pe_rings = [nc.sync, nc.sync, nc.sync, nc.gpsimd] * 2
            di = 0
            for pi in range(npe):
                emit_pe_chunk(*pe_units[pi], pe_rings[pi])
                dt = (pi + 1) * ndve // npe
                while di < dt:
                    u = dve_units[di]
                    emit_dve_compute(dve_tiles[di], u[1], u[2], u[3], u[4])
                    di += 1
            emit_fb_group(0)
            emit_fb_group(1)
            nc.vector.tensor_copy(sg_bf[:], sg[:])
            nc.sync.dma_start(
                blob_in[0, OF_C:OF_C + SZ_C].rearrange("(fb f) -> f fb", fb=nfb),
                sg_bf[:])
            nc.sync.dma_start(
                blob_in[0, OF_D:OF_D + SZ_D].rearrange("(fb f) -> f fb", fb=nfb),
                b_bf[:])
            nc.gpsimd.collective_compute(
                "AllGather", mybir.AluOpType.bypass,
                replica_groups=[list(range(n_cores))],
                ins=[blob_in.opt()], outs=[blob_out.opt()])

            # keep the PE's HAM clock warm across the collective window:
            # a serial junk chain (matmul -> vector copy -> matmul ...)
            # paced at ~1.5us per link, ending roughly when the AG lands.
            wch = pp.tile([128, 128], BF16, tag="wch")
            nc.vector.tensor_copy(wch[:], eye_sb[:])
            for _ in range(26):
                pj = ps_prep.tile([128, 128], F32, tag="prep")
                nc.tensor.matmul(pj[:], eye_sb[:], wch[:],
                                 start=True, stop=True)
                wch = pwp.tile([128, 128], BF16, tag="wchx")
                nc.vector.tensor_copy(wch[:], pj[:])

            # ---- post-AG loads: global stationary tiles, per f-block ----
            sg_g = pp.tile([128, nfb_g], BF16, tag="sgg")
            b_gbf = pp.tile([128, nfb_g], BF16, tag="bgbf")
            for fb in range(nfb):
                nc.gpsimd.dma_start(
                    sg_g.rearrange("ff (r fb) -> ff r fb",
                                   r=n_cores)[:, :, fb],
                    blob_out[:, OF_C + fb * 128:OF_C + (fb + 1) * 128]
                    .rearrange("r ff -> ff r"))
                nc.gpsimd.dma_start(
                    b_gbf.rearrange("ff (r fb) -> ff r fb",
                                    r=n_cores)[:, :, fb],
                    blob_out[:, OF_D + fb * 128:OF_D + (fb + 1) * 128]
                    .rearrange("r ff -> ff r"))
            b_g = pp.tile([128, nfb_g], F32, tag="bg")
            nc.vector.tensor_copy(b_g[:], b_gbf[:])

            # global fb index g = r*nfb + fb
            w_inT_g = pp.tile([128, f], BF16, tag="winTg")
            w_outg_g = pp.tile([128, f], BF16, tag="woutgg")
            w_inT_gv = w_inT_g.rearrange("z (r fb ff) -> z r fb ff",
                                         r=n_cores, fb=nfb)
            w_outg_gv = w_outg_g.rearrange("ff (r fb zz) -> ff r fb zz",
                                           r=n_cores, fb=nfb)
            for fb in range(nfb):
                nc.sync.dma_start(
                    w_inT_gv[:, :, fb, :],
                    blob_out[:, 0:SZ_A]
                    .rearrange("r (z fb ff) -> z r fb ff",
                               z=128, fb=nfb)[:, :, fb, :])
                nc.scalar.dma_start(
                    w_outg_gv[:, :, fb, :],
                    blob_out[:, OF_B + fb * 128 * 128:
                             OF_B + (fb + 1) * 128 * 128]
                    .rearrange("r (ff zz) -> ff r zz", ff=128))

            # cneg = -sum_f sg / F  (global)
            sgs = pp.tile([128, 1], F32, tag="sgs")
            nc.vector.tensor_reduce(sgs[:], sg_g[:], mybir.AxisListType.X, ADD)
            csum = pp.tile([1, 1], F32, tag="csum")
            nc.gpsimd.tensor_reduce(csum[:], sgs[:], mybir.AxisListType.XYZWC,
                                    ADD)
            cneg = pp.tile([1, 1], F32, tag="cneg")
            nc.scalar.mul(cneg[:], csum[:], -1.0 / f)

            # ---- stage B: own batch shard x full F ----------------------
            for j in range(nbc):
                b0 = j * bc
                pdz = ps_dz.tile([128, bc], F32, tag="pdz")
                pt2 = ps_t2.tile([1, bc], F32, tag="pt2")
                hs = [None] * nfb_g
                h2s = [None] * nfb_g

                def emit_ph(fb):
                    ph = ps_h.tile([128, bc], F32, tag="ph")
                    nc.tensor.matmul(ph[:],
                                     w_inT_g[:, fb * 128:(fb + 1) * 128],
                                     zt_sb[:, b0:b0 + bc],
                                     start=True, stop=True)
                    h_bf = hp.tile([128, bc], BF16, tag="hbf")
                    nc.scalar.activation(h_bf[:], ph[:],
                                         mybir.ActivationFunctionType.Tanh,
                                         bias=b_g[:, fb:fb + 1])
                    h2_bf = h2p.tile([128, bc], BF16, tag="h2bf")
                    nc.vector.tensor_mul(h2_bf[:], h_bf[:], h_bf[:])
                    hs[fb] = h_bf
                    h2s[fb] = h2_bf

                def emit_acc(fb):
                    nc.tensor.matmul(pdz[:],
                                     w_outg_g[:, fb * 128:(fb + 1) * 128],
                                     hs[fb][:],
                                     start=(fb == 0), stop=(fb == nfb_g - 1))
                    nc.tensor.matmul(pt2[:], sg_g[:, fb:fb + 1], h2s[fb][:],
                                     start=(fb == 0), stop=(fb == nfb_g - 1))

                for fb in range(nfb_g):
                    emit_ph(fb)
                    if fb >= 1:
                        emit_acc(fb - 1)
                emit_acc(nfb_g - 1)

                dz_sb = wp.tile([128, bc], F32, tag="dzsb")
                nc.scalar.mul(dz_sb[:], pdz[:], 1.0 / f)
                nc.sync.dma_start(out_ap[0:Z, b0:b0 + bc], dz_sb[:])
                tr_sb = wp.tile([1, bc], F32, tag="trsb")
                nc.scalar.activation(
                    tr_sb[:], pt2[:],
                    mybir.ActivationFunctionType.Identity,
                    bias=cneg[0:1, 0:1], scale=1.0 / f)
                nc.gpsimd.dma_start(out_ap[Z:Z + 1, b0:b0 + bc], tr_sb[:])

    nc.compile()
    return nc


def host_prep(t, z_and_logpz, W1, B1, W2, B2, W3_win, b3_win,
              W3_wout, b3_wout, W3_b, b3_b, W3_gate, b3_gate,
              n_cores=N_CORES, b=B, f=F, pe_cols=PE_COLS):
    """Shard + lay out the numpy inputs into per-core in_maps."""
    fl = f // n_cores
    nfb = fl // 128
    rows = fl * Z
    rows_pe = pe_cols * 128
    n_pe_chunks = rows_pe // CHUNK_R

    dcc2 = 2 * DVE_CC   # f-columns per DVE DMA pair

    def pack_pe(x):  # [rows_pe, N] -> [128, nch*2*chunk_r], chunk-major
        a = np.ascontiguousarray(x.T)                 # [256, rows_pe]
        return np.ascontiguousarray(
            a.reshape(2, 128, n_pe_chunks, CHUNK_R).transpose(1, 2, 0, 3)
            .reshape(128, n_pe_chunks * 2 * CHUNK_R))

    def pack_nat(x):  # [rows_dve, N] -> [nch*128, dcc2*N], partition-contig
        nch = x.shape[0] // (dcc2 * 128)
        return np.ascontiguousarray(
            x.reshape(nch, dcc2, 128, N).transpose(0, 2, 1, 3)
            .reshape(nch * 128, dcc2 * N))

    def col8(x):  # [4, 256] -> [128, 8] with col = k*2 + nb
        return np.ascontiguousarray(
            np.asarray(x, np.float32).reshape(4, 2, 128).transpose(2, 0, 1)
            .reshape(128, 8))

    t_in = np.ascontiguousarray(
        np.broadcast_to(np.asarray(t, np.float32).reshape(1, 1), (128, 1)))
    parc = np.ascontiguousarray(np.concatenate(
        [col8(np.asarray(W1, np.float32)[:, :, 0]), col8(B1), col8(B2)],
        axis=1))
    # lhsT tile for h1 net: [m128, (k4, mb, n)] = W2[k4, n, mb*128+m128]
    w2tc = np.ascontiguousarray(
        np.asarray(W2, np.float32).transpose(0, 2, 1)        # [k, m, n]
        .reshape(4, 2, 128, 256).transpose(2, 0, 1, 3).reshape(128, 2048)).astype(BF)
    w3win_bf = np.asarray(W3_win, np.float32).astype(BF)
    w3wout_bf = np.asarray(W3_wout, np.float32).astype(BF)
    w3b_bf = np.asarray(W3_b, np.float32).astype(BF)
    w3gate_bf = np.asarray(W3_gate, np.float32).astype(BF)
    b3win = np.asarray(b3_win, np.float32)
    b3wout = np.asarray(b3_wout, np.float32)
    b3b = np.asarray(b3_b, np.float32)
    b3gate = np.asarray(b3_gate, np.float32)
    z = np.asarray(z_and_logpz, np.float32)[:, :Z]
    ztb = np.ascontiguousarray(z.T).astype(BF)
    eye = np.eye(128, dtype=np.float32).astype(BF)
    bl = b // n_cores

    in_maps = []
    for k in range(n_cores):
        r0 = k * rows
        f0 = k * fl
        in_maps.append({
            "t": t_in, "parc": parc, "w2tc": w2tc,
            "w3winT_sl": pack_pe(w3win_bf[r0:r0 + rows_pe]),
            "w3woutT_sl": pack_pe(w3wout_bf[r0:r0 + rows_pe]),
            "w3winN_sl": pack_nat(w3win_bf[r0 + rows_pe:r0 + rows]),
            "w3woutN_sl": pack_nat(w3wout_bf[r0 + rows_pe:r0 + rows]),
            "b3win_c": np.ascontiguousarray(
                b3win[r0:r0 + rows].reshape(fl, 128).T),
            "b3wout_c": np.ascontiguousarray(
                b3wout[r0:r0 + rows].reshape(fl, 128).T),
            "w3bT_sl": np.ascontiguousarray(w3b_bf[f0:f0 + fl].T),
            "w3gateT_sl": np.ascontiguousarray(w3gate_bf[f0:f0 + fl].T),
            "b3b_c": np.ascontiguousarray(b3b[f0:f0 + fl].reshape(nfb, 128).T),
            "b3gate_c": np.ascontiguousarray(
                b3gate[f0:f0 + fl].reshape(nfb, 128).T),
            "ztb_sl": np.ascontiguousarray(ztb[:, k * bl:(k + 1) * bl]),
            "eyeb": eye,
        })
    return in_maps


_NC_CACHE = {}


def kernel(**inputs) -> np.ndarray:
    _ensure_ntff_hook()
    from concourse import bass_utils

    key = "full"
    if key not in _NC_CACHE:
        _NC_CACHE[key] = build_module()
    nc = _NC_CACHE[key]

    in_maps = host_prep(**inputs)
    res = bass_utils.run_bass_kernel_spmd(nc, in_maps, list(range(N_CORES)))
    bl = B // N_CORES
    out = np.empty((B, Z + 1), np.float32)
    for k in range(N_CORES):
        out[k * bl:(k + 1) * bl, :] = res.results[k]["out"].T
    return out
